# revision 22
# baseline (speedup 1.0000x reference)
"""GATv2 message-passing kernel for 8 Trainium2 NeuronCores (Bass/Tile).

Strategy (edge parallelism over receiver-sorted edges):
  * Sort edges by receiver on the host; receivers fall into 128-node blocks.
  * Deal the blocks to the 8 cores balanced by edge count, so every core owns
    complete receiver segments -> the segment softmax/sums are core-local and
    no collective is needed.  Each core returns its blocks' output rows and
    the host reassembles them.
  * All heavy tensors are bf16: node features, h scratch, gathered sender
    rows, edge features, and the receiver one-hot matrices (built on the
    host and streamed from HBM in both orientations, replacing on-device
    is_equal work).  PE matmuls run at 1 cycle/row in bf16 vs 4 for fp32.
  * Per chunk of up to 1024 edges: indirect-DMA gather of h[senders],
    x = es^T + Hb@GT + We@ef accumulated in PSUM, mish via exp/ln/tanh on
    the ACT engine, logits via small PE matmuls, softmax denominators and
    weighted message sums via one-hot PE matmuls.
  * ACT table loads are amortized by batching K positions per table phase:
    stage 1 (exp+ln, one table set) for all chunks of the batch, then
    stage 2 (tanh + the logit exp) -> 2 table loads per K positions.
  * Softmax is computed without the segment-max shift: logits are clamped
    to <= 60 before exp (guards padding-edge junk); exp(60) is finite in
    fp32/bf16 and real logits are far below the clamp, so the result is
    mathematically identical.
"""

import ml_dtypes
import numpy as np

import concourse.bass as bass
import concourse.bacc as bacc
import concourse.tile as tile
from concourse import mybir
from concourse.bass_utils import run_bass_kernel_spmd

F32 = mybir.dt.float32
BF16 = mybir.dt.bfloat16
I32 = mybir.dt.int32
AF = mybir.ActivationFunctionType
OP = mybir.AluOpType

# The act-table-load pass assigns each activation the FIRST table set whose
# function list contains it.  By default Exp->exp_and_others and
# Ln->natural_log, which puts the exp/ln pair in different sets and forces a
# table reload between them.  Emptying those two sets (ids preserved) makes
# Exp and Ln co-resolve to natural_log_exp_and_others and Tanh to
# sigmoid_and_others: with K-position stage batching this costs 2 reloads
# per K positions.
import concourse.hw_specs as _hw_specs
import concourse.bacc as _bacc_mod

if not hasattr(_hw_specs, "_gat_orig"):
    _hw_specs._gat_orig = _hw_specs.get_activation_tables

    def _gat_patched(arch):
        t = dict(_hw_specs._gat_orig(arch))
        for k in ("exp_and_others", "natural_log"):
            if k in t:
                t[k] = set()
        return t

    _hw_specs.get_activation_tables = _gat_patched
    _bacc_mod.get_activation_tables = _gat_patched

N_NODES = 50000
N_EDGES = 800000
IN_DIM = 256
EDGE_DIM = 64
EMBED = 128
HEADS = 8
HEAD_DIM = EMBED // HEADS
P = 128
NCORES = 8
CHUNK_G = 8        # max groups (of 128 edges) per processing chunk
K_BATCH = 3        # positions per ACT-table phase batch
GATHER_MODE = "swdge"  # "swdge" (dma_gather) or "indirect" (InstDMACopy)
LOGIT_CLAMP = 60.0
UW = EMBED + HEADS  # rb columns: [weighted message | exp(logit)]
ONEHOT_DT = BF16
ONEHOT_NP = ml_dtypes.bfloat16
BF16_NP = ml_dtypes.bfloat16


# ---------------------------------------------------------------- host plan

def _chunk_sizes(g):
    """Split g groups into balanced chunks of at most CHUNK_G groups."""
    nch = -(-g // CHUNK_G)
    base, rem = divmod(g, nch)
    return [base + (i < rem) for i in range(nch)]


def _plan(receivers, senders, n_nodes, ncores):
    """Sort edges by receiver, then by sender within each 128-node receiver
    block (so gathers use monotone addresses and fit int16 index windows);
    deal blocks to cores balanced by edge count; pad every (core, position)
    to a common group count; pick per-(position, chunk) gather base offsets
    shared by all cores."""
    order = np.argsort(receivers, kind="stable").astype(np.int64)
    r_s = receivers[order].astype(np.int64)
    nb = -(-n_nodes // P)
    npos = -(-nb // ncores)
    nb_pad = npos * ncores
    n_pad = nb_pad * P
    cnt = np.bincount(r_s // P, minlength=nb_pad).astype(np.int64)
    estart = np.zeros(nb_pad, np.int64)
    estart[1:] = np.cumsum(cnt)[:-1]
    for b in range(nb_pad):
        e0, c = int(estart[b]), int(cnt[b])
        if c > 1:
            seg = order[e0:e0 + c]
            order[e0:e0 + c] = seg[np.argsort(senders[seg], kind="stable")]
    r_s = receivers[order].astype(np.int64)
    gcnt = np.maximum(-(-cnt // P), 1)
    deal = np.argsort(-gcnt, kind="stable")
    blocks = deal.reshape(npos, ncores)  # blocks[pos, core] -> block id
    gpos = gcnt[blocks].max(axis=1)      # groups per position (same all cores)
    goff = np.zeros(npos, np.int64)
    goff[1:] = np.cumsum(gpos)[:-1]
    # per-(position, chunk) gather windows, uniform across cores; grow the
    # chunk count per position until every chunk's sender window fits int16
    chunks = []
    bases = []
    for pos in range(npos):
        g = int(gpos[pos])
        scs = []
        for core in range(ncores):
            b = int(blocks[pos, core])
            e0, c = int(estart[b]), int(cnt[b])
            scs.append(senders[order[e0:e0 + c]].astype(np.int64))
        nch0 = -(-g // CHUNK_G)
        for nch in range(nch0, g + 1):
            base, rem = divmod(g, nch)
            szs = [base + (i < rem) for i in range(nch)]
            lo = np.full(nch, np.iinfo(np.int64).max)
            hi = np.zeros(nch, np.int64)
            for sc in scs:
                r0 = 0
                for ch, gc in enumerate(szs):
                    part = sc[r0 * P:(r0 + gc) * P]
                    r0 += gc
                    if part.size:
                        lo[ch] = min(lo[ch], int(part.min()))
                        hi[ch] = max(hi[ch], int(part.max()))
            lo[lo > hi] = 0  # all-padding chunks
            if (hi - lo).max() < 32768:
                break
        else:
            raise AssertionError(f"no chunking fits int16 at pos {pos}")
        chunks.append(szs)
        bases.append([int(x) for x in lo])
    return dict(order=order, r_s=r_s, cnt=cnt, estart=estart, blocks=blocks,
                gpos=gpos, goff=goff, gtot=int(gpos.sum()),
                ecap=int(gpos.sum()) * P, npos=npos, nb_pad=nb_pad,
                chunks=chunks, bases=bases, n_pad=n_pad)


def _host_inputs(plan, node_features, edge_features, W_kernel, W_bias,
                 We_kernel, We_bias, a, senders):
    """Build the per-core input maps (all numpy, no math beyond transposes)."""
    npos, gtot, ecap = plan["npos"], plan["gtot"], plan["ecap"]
    n_pad = plan["nb_pad"] * P
    n_nodes, in_dim = node_features.shape
    heads, head_dim = a.shape
    embed = heads * head_dim
    edge_dim = edge_features.shape[1]

    nfT = np.zeros((in_dim, n_pad), BF16_NP)
    nfT[:, :n_nodes] = node_features.T.astype(BF16_NP)
    We_aug = np.concatenate(
        [We_kernel, (We_bias + 2.0 * W_bias)[None, :]], axis=0
    ).astype(BF16_NP)
    A_blk = np.zeros((embed, heads), BF16_NP)
    for h in range(heads):
        A_blk[h * head_dim:(h + 1) * head_dim, h] = a[h].astype(BF16_NP)
    Wb_rep = np.tile(W_bias[None, :], (P, 1)).astype(np.float32)
    identity = np.eye(P, dtype=BF16_NP)
    iota_row8 = np.tile(np.arange(P, dtype=BF16_NP)[None, :], (P, CHUNK_G))
    iota_col = np.arange(P, dtype=np.float32)[:, None].copy()
    ones_row = np.ones((1, P), BF16_NP)

    efT_all = np.ascontiguousarray(
        edge_features[plan["order"]].T.astype(BF16_NP))  # [64, E]
    s_sorted = senders[plan["order"]].astype(np.int64)
    rloc_all = (plan["r_s"] - (plan["r_s"] // P) * P).astype(np.int64)

    shared = {
        "nfT": nfT, "W": W_kernel.astype(BF16_NP), "We_aug": We_aug,
        "A_blk": A_blk, "Wb_rep": Wb_rep, "identity": identity,
        "iota_row8": iota_row8, "iota_col": iota_col, "ones_row": ones_row,
    }
    in_maps = []
    for core in range(NCORES):
        senders16 = np.zeros((P, gtot * 8), np.int16)
        senders32 = np.full((P, gtot), n_pad - 1, np.int32)
        efTa = np.zeros((edge_dim + 1, ecap), BF16_NP)
        rloc_flat = np.full((1, ecap), 200.0, BF16_NP)
        rloc_col = np.full((P, gtot), 200.0, BF16_NP)
        blocknodes = np.zeros((P, npos), np.int32)
        for pos in range(npos):
            b = int(plan["blocks"][pos, core])
            g0 = int(plan["goff"][pos])
            c = int(plan["cnt"][b])
            e0 = int(plan["estart"][b])
            blocknodes[:, pos] = b * P + np.arange(P)
            col0 = g0 * P
            efTa[:edge_dim, col0:col0 + c] = efT_all[:, e0:e0 + c]
            efTa[edge_dim, col0:col0 + c] = 1.0
            rl = rloc_all[e0:e0 + c]
            rloc_flat[0, col0:col0 + c] = rl.astype(BF16_NP)
            ei = np.arange(c)
            senders32[ei % P, g0 + ei // P] = s_sorted[e0:e0 + c]
            rloc_col[ei % P, g0 + ei // P] = rl.astype(BF16_NP)
            r0 = 0
            for ch, gc in enumerate(plan["chunks"][pos]):
                s_ch = gc * P
                base = plan["bases"][pos][ch]
                tmp_s = np.full(s_ch, base, np.int64)  # pads -> row `base`
                nreal = min(max(c - r0 * P, 0), s_ch)
                tmp_s[:nreal] = s_sorted[e0 + r0 * P:e0 + r0 * P + nreal]
                rel = (tmp_s - base).astype(np.int16)
                blk16 = np.tile(rel.reshape(s_ch // 16, 16).T, (8, 1))
                cb = (g0 + r0) * 8
                senders16[:, cb:cb + s_ch // 16] = blk16
                r0 += gc
        m = dict(shared)
        m.update({"senders16": senders16, "senders32": senders32,
                  "efTa": efTa, "rloc_flat": rloc_flat,
                  "rloc_col": rloc_col, "blocknodes": blocknodes})
        in_maps.append(m)
    return in_maps


# ---------------------------------------------------------------- bass build

def _build(plan, n_pad, in_dim, edge_dim, embed, heads, repeat=1,
           parts="full"):
    head_dim = embed // heads
    npos, gtot, ecap = plan["npos"], plan["gtot"], plan["ecap"]
    gpos, goff = plan["gpos"], plan["goff"]
    gmax = int(gpos.max())
    LW = gmax * heads  # per-position logit columns (padded)

    nc = bacc.Bacc("TRN2")
    t_nfT = nc.dram_tensor("nfT", [in_dim, n_pad], BF16, kind="ExternalInput")
    t_W = nc.dram_tensor("W", [in_dim, embed], BF16, kind="ExternalInput")
    t_We = nc.dram_tensor("We_aug", [edge_dim + 1, embed], BF16,
                          kind="ExternalInput")
    t_A = nc.dram_tensor("A_blk", [embed, heads], BF16, kind="ExternalInput")
    t_Wb = nc.dram_tensor("Wb_rep", [P, embed], F32, kind="ExternalInput")
    t_id = nc.dram_tensor("identity", [P, P], BF16, kind="ExternalInput")
    t_s16 = nc.dram_tensor("senders16", [P, gtot * 8], mybir.dt.int16,
                           kind="ExternalInput")
    t_s32 = nc.dram_tensor("senders32", [P, gtot], I32,
                           kind="ExternalInput")
    t_efT = nc.dram_tensor("efTa", [edge_dim + 1, ecap], BF16,
                           kind="ExternalInput")
    t_rlf = nc.dram_tensor("rloc_flat", [1, ecap], BF16, kind="ExternalInput")
    t_rlc = nc.dram_tensor("rloc_col", [P, gtot], BF16, kind="ExternalInput")
    t_ior8 = nc.dram_tensor("iota_row8", [P, CHUNK_G * P], BF16,
                            kind="ExternalInput")
    t_ioc = nc.dram_tensor("iota_col", [P, 1], F32, kind="ExternalInput")
    t_ones = nc.dram_tensor("ones_row", [1, P], BF16, kind="ExternalInput")
    t_bn = nc.dram_tensor("blocknodes", [P, npos], I32, kind="ExternalInput")
    t_out = nc.dram_tensor("out", [npos * P, embed], BF16,
                           kind="ExternalOutput")
    t_h = nc.dram_tensor("h_scratch", [n_pad, embed], BF16, kind="Internal")

    with tile.TileContext(nc) as tc:
        with tc.tile_pool(name="const", bufs=1) as cp:
            def cload(t, shape):
                s = cp.tile(shape, t.dtype, tag=f"c_{t.name}")
                nc.sync.dma_start(out=s[:], in_=t[:])
                return s

            W0 = cp.tile([P, embed], BF16)
            nc.sync.dma_start(out=W0[:], in_=t_W[0:P, :])
            W1 = cp.tile([P, embed], BF16)
            nc.sync.dma_start(out=W1[:], in_=t_W[P:2 * P, :])
            We = cload(t_We, [edge_dim + 1, embed])
            Ab = cload(t_A, [embed, heads])
            Wb = cload(t_Wb, [P, embed])
            idn = cload(t_id, [P, P])
            rlc = cload(t_rlc, [P, gtot])
            ior8 = cload(t_ior8, [P, CHUNK_G * P])
            ioc = cload(t_ioc, [P, 1])
            ones = cload(t_ones, [1, P])
            if GATHER_MODE == "swdge":
                s16 = cload(t_s16, [P, gtot * 8])
            else:
                s32 = cload(t_s32, [P, gtot])
            bn = cload(t_bn, [P, npos])

            for _rep in range(repeat):
              # ---------------- phase A: h = nf @ W (no bias) ----------------
              with tc.tile_pool(name=f"ha{_rep}", bufs=6) as hap, \
                      tc.tile_pool(name=f"haps{_rep}", bufs=2,
                                   space="PSUM") as hpp:
                  HW_ = 4 * P  # nodes per sweep
                  for nt in range(n_pad // HW_):
                      na = hap.tile([P, HW_], BF16, tag="nfT0")
                      nc.sync.dma_start(
                          out=na[:],
                          in_=t_nfT[0:P, nt * HW_:(nt + 1) * HW_])
                      nb_t = hap.tile([P, HW_], BF16, tag="nfT1")
                      nc.sync.dma_start(
                          out=nb_t[:],
                          in_=t_nfT[P:2 * P, nt * HW_:(nt + 1) * HW_])
                      hp = hpp.tile([P, HW_], F32, tag="hps")
                      for t in range(HW_ // P):
                          nc.tensor.matmul(hp[:, t * embed:(t + 1) * embed],
                                           lhsT=na[:, t * P:(t + 1) * P],
                                           rhs=W0[:], start=True, stop=False)
                          nc.tensor.matmul(hp[:, t * embed:(t + 1) * embed],
                                           lhsT=nb_t[:, t * P:(t + 1) * P],
                                           rhs=W1[:], start=False, stop=True)
                      hstage = hap.tile([P, HW_], BF16, tag="hstage")
                      nc.scalar.activation(out=hstage[:], in_=hp[:],
                                           func=AF.Copy)
                      out_view = bass.AP(
                          t_h[:].tensor, nt * HW_ * embed,
                          [[embed, P], [P * embed, HW_ // P], [1, embed]])
                      nc.sync.dma_start(out=out_view, in_=hstage[:])

              tc.strict_bb_all_engine_barrier()
              if parts == "a":
                  continue

              # ---------------- phase B: edge processing -------------------
              maxch = max(len(c) for c in plan["chunks"])
              nbuf = K_BATCH * maxch + 1
              with tc.tile_pool(name=f"eb{_rep}", bufs=4) as ep, \
                      tc.tile_pool(name=f"es{_rep}", bufs=nbuf) as esp, \
                      tc.tile_pool(name=f"xs{_rep}", bufs=nbuf) as xsp, \
                      tc.tile_pool(name=f"sps{_rep}", bufs=nbuf) as spp, \
                      tc.tile_pool(name=f"ebp{_rep}", bufs=2,
                                   space="PSUM") as pp, \
                      tc.tile_pool(name=f"rpp{_rep}", bufs=1,
                                   space="PSUM") as rp, \
                      tc.tile_pool(name=f"lgp{_rep}", bufs=1,
                                   space="PSUM") as lp, \
                      tc.tile_pool(name=f"ups{_rep}", bufs=1,
                                   space="PSUM") as up:
                  for kb0 in range(0, npos, K_BATCH):
                      kposs = list(range(kb0, min(kb0 + K_BATCH, npos)))
                      lgb = lp.tile([P, len(kposs) * LW], F32, tag="lgb")
                      Hb_t = {}
                      es_t = {}
                      xc_t = {}
                      sp_t = {}
                      # ---- stage 1: x, exp(x), softplus(x) ----------------
                      for ki, pos in enumerate(kposs):
                          g0 = int(goff[pos])
                          szs = plan["chunks"][pos]
                          Hb = ep.tile([P, embed], BF16, tag="Hb", bufs=8)
                          Hb_t[pos] = Hb
                          nc.gpsimd.indirect_dma_start(
                              out=Hb[:], out_offset=None, in_=t_h[:],
                              in_offset=bass.IndirectOffsetOnAxis(
                                  ap=bn[:, pos:pos + 1], axis=0))
                          r0 = 0
                          for ch, gc in enumerate(szs):
                              s = gc * P
                              col0 = (g0 + r0) * P
                              es = esp.tile([P, CHUNK_G * P], BF16, tag="es")
                              es_t[(pos, ch)] = es
                              if GATHER_MODE == "swdge":
                                  base = plan["bases"][pos][ch]
                                  rows = min(n_pad - base, 32768)
                                  cb = (g0 + r0) * 8
                                  nc.gpsimd.dma_gather(
                                      out_ap=es[:, :s].rearrange(
                                          "p (j e) -> p j e", e=embed),
                                      in_ap=t_h[base:base + rows, :],
                                      idxs_ap=s16[:, cb:cb + s // 16],
                                      num_idxs=s, num_idxs_reg=s,
                                      elem_size=embed)
                              else:
                                  nc.gpsimd.indirect_dma_start(
                                      out=es[:, :s].rearrange(
                                          "p (j e) -> p j e", e=embed),
                                      out_offset=None, in_=t_h[:],
                                      in_offset=bass.IndirectOffsetOnAxis(
                                          ap=s32[:, g0 + r0:g0 + r0 + gc],
                                          axis=0))
                              if parts == "ag":
                                  r0 += gc
                                  continue
                              ef = ep.tile([edge_dim + 1, CHUNK_G * P], BF16,
                                           tag="ef")
                              nc.sync.dma_start(
                                  out=ef[:, :s],
                                  in_=t_efT[:, col0:col0 + s])
                              rrow = ep.tile([1, CHUNK_G * P], BF16,
                                             tag="rrow")
                              nc.sync.dma_start(
                                  out=rrow[0:1, :s],
                                  in_=t_rlf[0:1, col0:col0 + s])
                              rep = rp.tile([P, CHUNK_G * P], F32, tag="rep")
                              for h0 in range(0, s, 512):
                                  hw = min(512, s - h0)
                                  nc.tensor.matmul(
                                      rep[:, h0:h0 + hw], lhsT=ones[:],
                                      rhs=rrow[0:1, h0:h0 + hw],
                                      start=True, stop=True)
                              GTt = ep.tile([P, CHUNK_G * P], BF16, tag="GT")
                              nc.vector.tensor_scalar(
                                  out=GTt[:, :s], in0=rep[:, :s],
                                  scalar1=ioc[:], scalar2=None,
                                  op0=OP.is_equal)
                              at = pp.tile([P, CHUNK_G * P], F32, tag="at")
                              for h0 in range(0, s, 512):
                                  hw = min(512, s - h0)
                                  nc.tensor.matmul(
                                      at[:, h0:h0 + hw], lhsT=We[:],
                                      rhs=ef[:, h0:h0 + hw],
                                      start=True, stop=False)
                                  nc.tensor.matmul(
                                      at[:, h0:h0 + hw], lhsT=Hb[:],
                                      rhs=GTt[:, h0:h0 + hw],
                                      start=False, stop=False)
                                  for j in range(h0 // P, (h0 + hw) // P):
                                      nc.tensor.matmul(
                                          at[:, j * P:(j + 1) * P],
                                          lhsT=es[:, j * P:(j + 1) * P],
                                          rhs=idn[:], start=False,
                                          stop=(j == (h0 + hw) // P - 1))
                              xc = xsp.tile([P, CHUNK_G * P], BF16, tag="xc")
                              xc_t[(pos, ch)] = xc
                              nc.vector.tensor_copy(out=xc[:, :s],
                                                    in_=at[:, :s])
                              vv = ep.tile([P, CHUNK_G * P], BF16, tag="vv")
                              nc.scalar.activation(out=vv[:, :s],
                                                   in_=at[:, :s], func=AF.Exp)
                              sp = spp.tile([P, CHUNK_G * P], BF16, tag="sp")
                              sp_t[(pos, ch)] = sp
                              nc.scalar.activation(out=sp[:, :s],
                                                   in_=vv[:, :s],
                                                   func=AF.Ln, bias=1.0)
                              r0 += gc
                      if parts == "ag":
                          continue
                      # ---- stage 2: tanh, mish, logits --------------------
                      for ki, pos in enumerate(kposs):
                          szs = plan["chunks"][pos]
                          lg0 = ki * LW
                          r0 = 0
                          for ch, gc in enumerate(szs):
                              s = gc * P
                              mi = ep.tile([P, CHUNK_G * P], BF16, tag="mish")
                              nc.scalar.activation(out=mi[:, :s],
                                                   in_=sp_t[(pos, ch)][:, :s],
                                                   func=AF.Tanh)
                              nc.vector.tensor_tensor(
                                  out=mi[:, :s], in0=xc_t[(pos, ch)][:, :s],
                                  in1=mi[:, :s], op=OP.mult)
                              for j in range(gc):
                                  nc.tensor.matmul(
                                      lgb[:, lg0 + (r0 + j) * heads:
                                          lg0 + (r0 + j + 1) * heads],
                                      lhsT=mi[:, j * P:(j + 1) * P], rhs=Ab[:],
                                      start=True, stop=True)
                              r0 += gc
                      # ---- batch end: clamp + exp of all logits -----------
                      lgw = lgb.shape[1]
                      lgc = ep.tile([P, lgw], BF16, tag="lgc", bufs=2)
                      nc.vector.tensor_scalar(
                          out=lgc[:], in0=lgb[:], scalar1=LOGIT_CLAMP,
                          scalar2=None, op0=OP.min)
                      exb = ep.tile([P, lgw], BF16, tag="exb", bufs=2)
                      nc.scalar.activation(out=exb[:], in_=lgc[:],
                                           func=AF.Exp)
                      # ---- stage 3: weighted scatter-accumulate -----------
                      for ki, pos in enumerate(kposs):
                          szs = plan["chunks"][pos]
                          g0 = int(goff[pos])
                          lg0 = ki * LW
                          Ups = up.tile([P, UW], F32, tag="U")
                          nch = len(szs)
                          r0 = 0
                          for ch, gc in enumerate(szs):
                              s = gc * P
                              col0 = (g0 + r0) * P
                              es = es_t[(pos, ch)]
                              Gt = ep.tile([P, CHUNK_G * P], BF16, tag="G")
                              gv = Gt[:, :s].rearrange("p (j c) -> p j c",
                                                       j=gc)
                              iv = ior8[:, :s].rearrange("p (j c) -> p j c",
                                                         j=gc)
                              rlcb = rlc[:, g0 + r0:g0 + r0 + gc
                                         ].to_broadcast([P, gc, P])
                              nc.vector.tensor_tensor(out=gv, in0=iv,
                                                      in1=rlcb,
                                                      op=OP.is_equal)
                              rb = ep.tile([P, CHUNK_G * UW], BF16, tag="rhsb")
                              rb3 = rb[:].rearrange("p (j c) -> p j c",
                                                    j=CHUNK_G)
                              ex_view = rb3[:, :gc, embed:UW]
                              exb_view = exb[:, lg0 + r0 * heads:
                                             lg0 + (r0 + gc) * heads
                                             ].rearrange(
                                  "p (j h) -> p j h", j=gc)
                              nc.vector.tensor_copy(out=ex_view, in_=exb_view)
                              m_view = rb3[:, :gc, 0:embed].rearrange(
                                  "p j (h w) -> p j h w", w=head_dim)
                              es_view = es[:, :s].rearrange(
                                  "p (j h w) -> p j h w", j=gc, w=head_dim)
                              ex_b = exb_view.to_broadcast(
                                  [P, gc, heads, head_dim])
                              meng = nc.vector if ch % 2 == 0 else nc.gpsimd
                              meng.tensor_tensor(out=m_view, in0=es_view,
                                                 in1=ex_b, op=OP.mult)
                              for j in range(gc):
                                  nc.tensor.matmul(
                                      Ups[:], lhsT=Gt[:, j * P:(j + 1) * P],
                                      rhs=rb[:, j * UW:(j + 1) * UW],
                                      start=(ch == 0 and j == 0),
                                      stop=(ch == nch - 1 and j == gc - 1))
                              r0 += gc
                          # ---- epilogue: out = U / max(denom, eps) + Wb ---
                          dn = ep.tile([P, heads], F32, tag="dn")
                          nc.vector.tensor_scalar(out=dn[:],
                                                  in0=Ups[:, embed:UW],
                                                  scalar1=1e-30, scalar2=None,
                                                  op0=OP.max)
                          rc = ep.tile([P, heads], F32, tag="rc")
                          nc.vector.reciprocal(rc[:], dn[:])
                          nd = ep.tile([P, embed], F32, tag="nodes")
                          ndv = nd[:].rearrange("p (h w) -> p h w", w=head_dim)
                          uv = Ups[:, 0:embed].rearrange("p (h w) -> p h w",
                                                         w=head_dim)
                          rcb = rc[:].to_broadcast([P, heads, head_dim])
                          nc.vector.tensor_tensor(out=ndv, in0=uv, in1=rcb,
                                                  op=OP.mult)
                          nd2 = ep.tile([P, embed], BF16, tag="nodes2")
                          nc.vector.tensor_tensor(out=nd2[:], in0=nd[:],
                                                  in1=Wb[:], op=OP.add)
                          nc.sync.dma_start(
                              out=t_out[pos * P:(pos + 1) * P, :],
                              in_=nd2[:])
              if _rep != repeat - 1:
                  tc.strict_bb_all_engine_barrier()
    nc.finalize()
    return nc


# ---------------------------------------------------------------- entry

def _run(node_features, edge_features, W_kernel, W_bias, We_kernel, We_bias,
         a, senders, receivers, trace=False):
    n_nodes, in_dim = node_features.shape
    heads, head_dim = a.shape
    embed = heads * head_dim
    edge_dim = edge_features.shape[1]
    plan = _plan(receivers, senders, n_nodes, NCORES)
    n_pad = plan["nb_pad"] * P
    in_maps = _host_inputs(plan, node_features, edge_features, W_kernel,
                           W_bias, We_kernel, We_bias, a, senders)
    nc = _build(plan, n_pad, in_dim, edge_dim, embed, heads)
    res = run_bass_kernel_spmd(nc, in_maps, core_ids=list(range(NCORES)),
                               trace=trace)
    # reassemble: core outputs are [npos*P, embed]; position rows -> blocks
    out = np.zeros((n_pad, embed), np.float32)
    for core in range(NCORES):
        o = np.asarray(res.results[core]["out"]).astype(np.float32)
        for pos in range(plan["npos"]):
            b = int(plan["blocks"][pos, core])
            out[b * P:(b + 1) * P] = o[pos * P:(pos + 1) * P]
    out = out[:n_nodes]
    # nodes with no incoming edges: reference segment_sum gives exactly 0
    deg = np.bincount(receivers.astype(np.int64), minlength=n_nodes)
    if (deg == 0).any():
        out[deg == 0] = 0.0
    return out, res


def kernel(node_features, edge_features, W_kernel, W_bias, We_kernel,
           We_bias, a, senders, receivers):
    node_features = np.asarray(node_features, np.float32)
    edge_features = np.asarray(edge_features, np.float32)
    W_kernel = np.asarray(W_kernel, np.float32)
    W_bias = np.asarray(W_bias, np.float32)
    We_kernel = np.asarray(We_kernel, np.float32)
    We_bias = np.asarray(We_bias, np.float32)
    a = np.asarray(a, np.float32)
    senders = np.asarray(senders, np.int32)
    receivers = np.asarray(receivers, np.int32)
    out, _ = _run(node_features, edge_features, W_kernel, W_bias, We_kernel,
                  We_bias, a, senders, receivers)
    return out
